# revision 37
# baseline (speedup 1.0000x reference)
"""AttentionRNN Trainium2 kernel: MHA + 2-layer Elman RNN + FC head.

Data-parallel over batch (4/core x 8 cores), weights replicated, fp16 PE,
fp32 PSUM.
  - RNN inner loop at the measured ~34ns LDW+MM pair issue floor (microbench:
    dependency-free N=4 pairs run 33.7ns; LDW reuse and fp8 weights do not
    lower it): k-outer matmul order, one full PSUM bank per pre tile,
    layer0/layer1 interleaved per step so each layer's tanh drains under the
    other layer's matmul burst. 17412 pairs => RNN phase ~630us at 97% PE
    occupancy; this is the structural floor for 2 recurrences/core.
  - attention output staged per batch contiguous; per-block DVE repack
    builds the [128, k, step, batch] rhs so the Wih0 input-projection
    matmuls stream contiguously.
  - Wih1's input projection reads the previous h0 block directly.
  - startup: b=0 x-transposes are first in the DMA queue, weights follow in
    first-use order; HAM warm-up stage 1 runs on memset data (no DMA dep)
    so the first projection starts ~7us in, warm.
  - softmax tail: one f32 DVE copy takes the AV rows AND the denominator row
    out of PSUM together, so the single pa accumulator bank frees after one
    reader and the next head's AV matmuls start a chain-hop earlier (-23us).
    The realignment copy for the denominator row must be partition-aligned
    before reciprocal_approx_fast (a partition-shifted recip input NaNs).
Rejected experiments (measured on hw): fp8-e4m3 RNN weights (no LDW gain,
0.10 rel err from recurrence amplification); 2-rank-AllGather layer pipeline
across core pairs (collectives work under axon, 18.7us/handoff round trip,
but 1-layer/core exposes the ~0.5us tanh round trip every step, capping the
gain below the comm risk); drip-feeding the next block's pre0 projection
between step bursts as tanh-wait cover, both scheduler-placed and force-
pinned via a tile_wait_until model-time ramp (the ramp DID control placement
— but foreign matmuls at burst boundaries disrupt the background-weight-
buffer swap and cost more than the ~120ns/layer-step wait they cover; the
scheduler's boundary bunching of the projection groups is the faster order).
"""

import os
import sys

try:
    import concourse  # noqa: F401
except ImportError:
    sys.path.insert(0, "/opt/trn_rl_repo")

import numpy as np
from contextlib import ExitStack

import concourse.bass as bass
import concourse.mybir as mybir
import concourse.tile as tile
from concourse import bacc
from concourse.bass import ds, ts
from concourse import bass_utils

N_CORES = 8
B, S, E, H, NH, HD = 32, 512, 512, 512, 8, 64
BC = B // N_CORES          # batch per core = 4
TOK = BC * S               # tokens per core = 2048
EC = E // 128              # 4 partition chunks
BLK = 16
NBLK = S // BLK

F16 = mybir.dt.float16
F32 = mybir.dt.float32
F8 = mybir.dt.float8e4
AF = mybir.ActivationFunctionType

# Optional weight pre-scale compensated inside the tanh activation's input
# scale. 1.0 = plain fp16 weights (fp8 was tried: LDWEIGHTS stays ~32ns
# regardless of dtype, and the recurrence amplifies the quantization to
# ~0.1 rel err, so fp16 it is).
WSCALE = 1.0


def build_nc(bfc_val: float):
    nc = bacc.Bacc("TRN2", target_bir_lowering=False, debug=False)

    x_d = nc.dram_tensor("x", [TOK, E], F16, kind="ExternalInput")
    w_names = ["wq", "wk", "wv", "wo", "wih0", "wih1", "whh0", "whh1"]
    w_dt = {n: F16 for n in w_names}
    w_d = {n: nc.dram_tensor(n, [128, EC, E], w_dt[n], kind="ExternalInput")
           for n in w_names}
    bq_d = nc.dram_tensor("bq", [128, EC], F32, kind="ExternalInput")
    bk_d = nc.dram_tensor("bk", [128, EC], F32, kind="ExternalInput")
    bo_d = nc.dram_tensor("bo", [128, EC], F32, kind="ExternalInput")
    b0_d = nc.dram_tensor("b0", [128, EC], F32, kind="ExternalInput")
    b1_d = nc.dram_tensor("b1", [128, EC], F32, kind="ExternalInput")
    wfc_d = nc.dram_tensor("wfc", [128, EC], F16, kind="ExternalInput")
    out_d = nc.dram_tensor("out", [BC, 1], F32, kind="ExternalOutput")

    with tile.TileContext(nc) as tc:
        with ExitStack() as ctx:
            consts = ctx.enter_context(tc.tile_pool(name="consts", bufs=1))
            w_sb = {}
            for n in w_names:
                w_sb[n] = consts.tile([128, EC, E], w_dt[n], tag=f"w_{n}",
                                      name=f"w_{n}")
            bq_sb = consts.tile([128, EC], F32, tag="bq")
            bk_sb = consts.tile([128, EC], F32, tag="bk")
            bo_sb = consts.tile([128, EC], F32, tag="bo")
            b0_sb = consts.tile([128, EC], F32, tag="b0")
            b1_sb = consts.tile([128, EC], F32, tag="b1")
            wfc_sb = consts.tile([128, EC], F16, tag="wfc")
            ones_sb = consts.tile([1, 64], F16, tag="ones")
            nc.vector.memset(ones_sb[:], 1.0)
            zeros_sb = consts.tile([128, EC, BC], F16, tag="zeros")
            nc.vector.memset(zeros_sb[:], 0.0)
            # attention output, staged per batch contiguous: [128, EC, BC, S]
            at_sb = consts.tile([128, EC, BC, S], F16, tag="at_all")

            # ---------------- Phase A: attention --------------------------
            with ExitStack() as actx, nc.named_scope("attn"):
                xt_p = actx.enter_context(tc.tile_pool(name="xt", bufs=2))
                qt_p = actx.enter_context(tc.tile_pool(name="qt", bufs=2))
                kt_p = actx.enter_context(tc.tile_pool(name="kt", bufs=2))
                va_p = actx.enter_context(tc.tile_pool(name="va", bufs=2))
                et_p = actx.enter_context(tc.tile_pool(name="et", bufs=4))
                cx_p = actx.enter_context(tc.tile_pool(name="cx", bufs=2))
                rp_p = actx.enter_context(tc.tile_pool(name="rp", bufs=2))
                avs_p = actx.enter_context(tc.tile_pool(name="avs", bufs=10))
                pj_p = actx.enter_context(tc.tile_pool(name="pj", bufs=2, space="PSUM"))
                ps_p = actx.enter_context(tc.tile_pool(name="ps", bufs=4, space="PSUM"))
                pa_p = actx.enter_context(tc.tile_pool(name="pa", bufs=1, space="PSUM"))
                pb_p = actx.enter_context(tc.tile_pool(name="pb", bufs=1, space="PSUM"))

                # DMA order: b=0's x transposes first (they gate the first
                # projection and are slow), then the small bias tensors, then
                # the weights in first-use order. Meanwhile HAM warm-up
                # stage 1 runs on memset data with no DMA dependency at all.
                xT0 = xt_p.tile([128, EC, E], F16, tag="xt")
                for m in range(EC):
                    nc.sync.dma_start_transpose(
                        xT0[:, m, :], x_d[ds(0, S), ts(m, 128)]
                    )
                for sb, d in [(bq_sb, bq_d), (bk_sb, bk_d), (bo_sb, bo_d),
                              (b0_sb, b0_d), (b1_sb, b1_d), (wfc_sb, wfc_d)]:
                    nc.sync.dma_start(sb[:], d[:])
                for n in ["wq", "wk", "wv", "wo", "wih0", "whh0", "wih1",
                          "whh1"]:
                    nc.sync.dma_start(w_sb[n][:], w_d[n][:])

                wu = ps_p.tile([128, 512], F32, tag="ps", name="warmup")
                for _ in range(150):
                    nc.tensor.matmul(
                        wu[0:64, 0:64], ones_sb[:, :], ones_sb[:, :],
                        start=True, stop=True,
                        skip_group_check=True,
                    )

                for b in range(BC):
                    if b == 0:
                        xT = xT0
                    else:
                        xT = xt_p.tile([128, EC, E], F16, tag="xt")
                        for m in range(EC):
                            nc.sync.dma_start_transpose(
                                xT[:, m, :], x_d[ds(b * S, S), ts(m, 128)]
                            )
                    if b == 0:
                        # warm-up stage 2: gated on xT chunk 0, spans the
                        # remaining transpose wait so the projections start warm
                        wu2 = ps_p.tile([128, 512], F32, tag="ps", name="warmup2")
                        for _ in range(70):
                            nc.tensor.matmul(
                                wu2[:, 0:128], xT[:, 0, 0:128], xT[:, 0, 0:128],
                                start=True, stop=True, skip_group_check=True,
                            )
                    QT = qt_p.tile([128, EC, S], F16, tag="qt")
                    KT = kt_p.tile([128, EC, S], F16, tag="kt")
                    for wname, bias_sb, dest in [("wq", bq_sb, QT), ("wk", bk_sb, KT)]:
                        for m in range(EC):
                            p = pj_p.tile([128, 512], F32, tag="pj")
                            for k in range(EC):
                                nc.tensor.matmul(
                                    p[:], w_sb[wname][:, k, ts(m, 128)], xT[:, k, :],
                                    start=(k == 0), stop=(k == EC - 1),
                                )
                            nc.vector.tensor_add(
                                out=dest[:, m, :], in0=p[:],
                                in1=bias_sb[:, m, None].to_broadcast((128, 512)),
                            )
                    VA = va_p.tile([128, EC, NH, HD + 1], F16, tag="va")
                    for n in range(EC):
                        pv = pj_p.tile([128, NH, HD], F32, tag="pj")
                        for k in range(EC):
                            nc.tensor.matmul(
                                pv[:], xT[:, k, ts(n, 128)], w_sb["wv"][:, k, :],
                                start=(k == 0), stop=(k == EC - 1),
                            )
                        nc.vector.tensor_copy(out=VA[:, n, :, 0:HD], in_=pv[:])
                        nc.vector.memset(VA[:, n, :, HD], 1.0)

                    CX = cx_p.tile([128, EC, S], F16, tag="cx")
                    for chn in range(NH // 2):
                        # score matmuls for the head pair emitted interleaved:
                        # 64-row stationaries land in row groups (0,0)/(64,0)
                        # and execute concurrently in different sub-arrays
                        ETp = [et_p.tile([128, EC, S], F16, tag="et",
                                         name=f"et{chn}_{i}") for i in range(2)]
                        for km in range(EC):
                            sps = []
                            for i, po in ((0, 0), (1, 64)):
                                sp = ps_p.tile([128, 512], F32, tag="ps")
                                nc.tensor.matmul(
                                    sp[:],
                                    KT[po:po + 64, chn, ts(km, 128)],
                                    QT[po:po + 64, chn, :],
                                    start=True, stop=True,
                                )
                                sps.append(sp)
                            for i in range(2):
                                nc.scalar.activation(ETp[i][:, km, :], sps[i][:],
                                                     AF.Exp)
                        for i in range(2):
                            h, po = 2 * chn + i, i * 64
                            ET = ETp[i]
                            av = pa_p.tile([128, 512], F32, tag="pa")
                            for km in range(EC):
                                nc.tensor.matmul(
                                    av[:HD + 1, :], VA[:, km, h, :], ET[:, km, :],
                                    start=(km == 0), stop=(km == EC - 1),
                                )
                            # single f32 copy takes the AV rows AND the
                            # denominator row out of PSUM in one op, so the
                            # pa bank frees after one reader and the next
                            # head's AV matmuls start a chain-hop earlier
                            avs = avs_p.tile([HD + 1, 512], F32, tag="avs",
                                             name=f"avs{h}")
                            nc.vector.tensor_copy(out=avs[:], in_=av[:HD + 1, :])
                            den = rp_p.tile([1, 512], F32, tag="den")
                            nc.vector.tensor_copy(out=den[:],
                                                  in_=avs[HD:HD + 1, :])
                            rp32 = rp_p.tile([1, 512], F32, tag="rp32")
                            nc.vector.reciprocal_approx_fast(rp32[:], den[:])
                            rp16 = rp_p.tile([1, 512], F16, tag="rp16")
                            nc.vector.tensor_copy(out=rp16[:], in_=rp32[:])
                            pb = pb_p.tile([64, 512], F32, tag="pb")
                            nc.tensor.matmul(pb[:], ones_sb[:], rp16[:, :],
                                             start=True, stop=True)
                            nc.vector.tensor_mul(
                                out=CX[po:po + 64, chn, :], in0=avs[:HD, :],
                                in1=pb[:]
                            )
                    for m in range(EC):
                        p = pj_p.tile([128, 512], F32, tag="pj")
                        for k in range(EC):
                            nc.tensor.matmul(
                                p[:], w_sb["wo"][:, k, ts(m, 128)], CX[:, k, :],
                                start=(k == 0), stop=(k == EC - 1),
                            )
                        nc.vector.tensor_add(
                            out=at_sb[:, m, b, :], in0=p[:],
                            in1=bo_sb[:, m, None].to_broadcast((128, 512)),
                        )

            # ---------------- Phase B: sequential RNN ---------------------
            # Per block j: SRC0[j] = DVE repack of at (contiguous rhs), pre0 =
            # Wih0 @ SRC0 into a full PSUM bank; pre1 = Wih1 @ H0B[j-1].
            # Per step: L0's 16 chunk-matmuls (k-outer), one tanh; L1's 16,
            # one tanh. Each layer's tanh drains under the other's matmuls.
            with ExitStack() as rctx, nc.named_scope("rnn"):
                src_p = rctx.enter_context(tc.tile_pool(name="src", bufs=3))
                h0b_p = rctx.enter_context(tc.tile_pool(name="h0b", bufs=2))
                h1b_p = rctx.enter_context(tc.tile_pool(name="h1b", bufs=2))
                os_p = rctx.enter_context(tc.tile_pool(name="os", bufs=1))
                pb0_p = rctx.enter_context(tc.tile_pool(name="pb0", bufs=3, space="PSUM"))
                pb1_p = rctx.enter_context(tc.tile_pool(name="pb1", bufs=2, space="PSUM"))
                pf_p = rctx.enter_context(tc.tile_pool(name="pf", bufs=1, space="PSUM"))

                h0_prev = zeros_sb[:, :, :]
                h1_prev = zeros_sb[:, :, :]
                h0b_done = None
                src_tiles = {}
                pre0_tiles = {}

                def make_src(jj):
                    t = src_p.tile([128, EC, BLK * BC], F16, tag="src",
                                   name=f"src{jj}")
                    nc.vector.tensor_copy(
                        out=t[:].rearrange("p k (s b) -> p k s b", b=BC),
                        in_=at_sb[:, :, :, ds(jj * BLK, BLK)].rearrange(
                            "p k b s -> p k s b"),
                    )
                    src_tiles[jj] = t

                def start_pre0(jj):
                    # full PSUM bank per pre tile (pad step axis 16 -> 32
                    # so every matmul write stays contiguous)
                    pre0_tiles[jj] = pb0_p.tile(
                        [128, EC, 2 * BLK * BC], F32, tag="pre0",
                        name=f"pre0_{jj}")

                def pre0_mm(jj, idx):
                    # One N=64 projection matmul (idx 0..15, k-outer). Blocks
                    # >= 2 drip one of these per step two blocks early: free
                    # PE work that pads the cover over both layers' tanh
                    # drain+sem chains (the ~120ns wait the first step-matmul
                    # of each layer-step otherwise absorbs).
                    # piece 0 is a self-contained group whose start=True
                    # clears the bank; the rest are stray accumulates like
                    # the step matmuls, so the Tile scheduler is free to
                    # leave them interleaved between step bursts instead of
                    # re-bunching them into one contiguous group.
                    k, m = idx // EC, idx % EC
                    nc.tensor.matmul(
                        pre0_tiles[jj][:, m, 0:BLK * BC],
                        w_sb["wih0"][:, k, ts(m, 128)],
                        src_tiles[jj][:, k, :],
                        start=(idx == 0), stop=(idx == 0),
                        skip_group_check=True,
                    )
                    if idx == EC * EC - 1:
                        nc.vector.tensor_add(
                            out=pre0_tiles[jj][:, :, 0:BLK * BC],
                            in0=pre0_tiles[jj][:, :, 0:BLK * BC],
                            in1=b0_sb[:, :, None].to_broadcast(
                                (128, EC, BLK * BC)),
                        )
                        src_tiles.pop(jj)

                for jj in (0, 1):
                    make_src(jj)
                    start_pre0(jj)
                    for idx in range(EC * EC):
                        pre0_mm(jj, idx)

                for j in range(NBLK + 1):
                    fill_j = j + 2 if j + 2 < NBLK else None
                    if fill_j is not None:
                        make_src(fill_j)
                        start_pre0(fill_j)
                    if j < NBLK:
                        pre0 = pre0_tiles.pop(j)
                        H0B = h0b_p.tile([128, EC, BLK * BC], F16, tag="h0b")
                    else:
                        H0B = None
                    if j >= 1:
                        pre1 = pb1_p.tile([128, EC, 2 * BLK * BC], F32, tag="pre1")
                        for k in range(EC):
                            for m in range(EC):
                                nc.tensor.matmul(
                                    pre1[:, m, 0:BLK * BC], w_sb["wih1"][:, k, ts(m, 128)],
                                    h0b_done[:, k, :],
                                    start=(m == 0 and k == 0),
                                    stop=(m == EC - 1 and k == EC - 1),
                                    skip_group_check=True,
                                )
                        nc.vector.tensor_add(
                            out=pre1[:, :, 0:BLK * BC], in0=pre1[:, :, 0:BLK * BC],
                            in1=b1_sb[:, :, None].to_broadcast((128, EC, BLK * BC)),
                        )
                        H1B = h1b_p.tile([128, EC, BLK * BC], F16, tag="h1b")
                    for t in range(BLK):
                        if j < NBLK:
                            for k in range(EC):
                                for m in range(EC):
                                    nc.tensor.matmul(
                                        pre0[:, m, ds(t * BC, BC)], w_sb["whh0"][:, k, ts(m, 128)],
                                        h0_prev[:, k, :], start=False, stop=False,
                                        skip_group_check=True,
                                    )
                            nc.scalar.activation(H0B[:, :, ds(t * BC, BC)],
                                                 pre0[:, :, ds(t * BC, BC)], AF.Tanh,
                                                 scale=1.0 / WSCALE)
                            h0_prev = H0B[:, :, ds(t * BC, BC)]
                        if j >= 1:
                            for k in range(EC):
                                for m in range(EC):
                                    nc.tensor.matmul(
                                        pre1[:, m, ds(t * BC, BC)], w_sb["whh1"][:, k, ts(m, 128)],
                                        h1_prev[:, k, :], start=False, stop=False,
                                        skip_group_check=True,
                                    )
                            nc.scalar.activation(H1B[:, :, ds(t * BC, BC)],
                                                 pre1[:, :, ds(t * BC, BC)], AF.Tanh,
                                                 scale=1.0 / WSCALE)
                            h1_prev = H1B[:, :, ds(t * BC, BC)]
                        if fill_j is not None:
                            pre0_mm(fill_j, t)
                    if j < NBLK:
                        h0b_done = H0B

                pf = pf_p.tile([BC, 1], F32, tag="pf")
                for k in range(EC):
                    nc.tensor.matmul(
                        pf[:], h1_prev[:, k, :], wfc_sb[:, k, None],
                        start=(k == 0), stop=(k == EC - 1),
                    )
                out_sb = os_p.tile([BC, 1], F32, tag="os")
                nc.scalar.activation(out_sb[:], pf[:], AF.Copy, bias=bfc_val)
                nc.sync.dma_start(out_d[:], out_sb[:])

    nc.compile()
    return nc


def _pack_w(wt: np.ndarray) -> np.ndarray:
    """[512,512] W.T (contraction-major) -> [128, EC, 512] fp16 chunk layout."""
    return np.ascontiguousarray(
        wt.reshape(EC, 128, E).transpose(1, 0, 2).astype(np.float16)
    )


def _pack_w8(wt: np.ndarray) -> np.ndarray:
    """Same chunk layout, scaled by WSCALE and cast to fp8-e4m3."""
    import ml_dtypes
    return np.ascontiguousarray(
        (wt * WSCALE).reshape(EC, 128, E).transpose(1, 0, 2)
        .astype(ml_dtypes.float8_e4m3fn)
    )


def _pack_b(b: np.ndarray) -> np.ndarray:
    return np.ascontiguousarray(b.reshape(EC, 128).T.astype(np.float32))


def prepare_inputs(inputs):
    x = np.asarray(inputs["x"], dtype=np.float32)
    Wq, bq = np.asarray(inputs["Wq"]), np.asarray(inputs["bq"])
    Wk, bk = np.asarray(inputs["Wk"]), np.asarray(inputs["bk"])
    Wv, bv = np.asarray(inputs["Wv"]), np.asarray(inputs["bv"])
    Wo, bo = np.asarray(inputs["Wo"]), np.asarray(inputs["bo"])
    Wih, bih = np.asarray(inputs["Wih"]), np.asarray(inputs["bih"])
    Whh, bhh = np.asarray(inputs["Whh"]), np.asarray(inputs["bhh"])
    Wfc, bfc = np.asarray(inputs["Wfc"]), np.asarray(inputs["bfc"])

    shared = {
        "wq": _pack_w(Wq.T / np.sqrt(np.float32(HD))),
        "wk": _pack_w(Wk.T),
        "wv": _pack_w(Wv.T),
        "wo": _pack_w(Wo.T),
        "wih0": _pack_w(Wih[0].T),
        "wih1": _pack_w(Wih[1].T),
        "whh0": _pack_w(Whh[0].T),
        "whh1": _pack_w(Whh[1].T),
        "bq": _pack_b(bq / np.sqrt(np.float32(HD))),
        "bk": _pack_b(bk),
        "bo": _pack_b(bo + Wo @ bv),
        "b0": _pack_b(bih[0] + bhh[0]),
        "b1": _pack_b(bih[1] + bhh[1]),
        "wfc": np.ascontiguousarray(
            Wfc[0].reshape(EC, 128).T.astype(np.float16)
        ),
    }
    x16 = x.astype(np.float16)
    in_maps = []
    for c in range(N_CORES):
        m = dict(shared)
        m["x"] = np.ascontiguousarray(
            x16[c * BC:(c + 1) * BC].reshape(TOK, E)
        )
        in_maps.append(m)
    return in_maps, float(bfc[0])


def run(inputs, trace=False):
    in_maps, bfc_val = prepare_inputs(inputs)
    nc = build_nc(bfc_val)
    if trace:
        _install_trace_shim()
        bass_utils.run_bass_kernel_spmd(
            nc, in_maps, core_ids=list(range(N_CORES)), trace=False
        )
    res = bass_utils.run_bass_kernel_spmd(
        nc, in_maps, core_ids=list(range(N_CORES)), trace=trace,
        trace_cores=list(range(N_CORES)) if trace else None,
    )
    out = np.concatenate([res.results[c]["out"] for c in range(N_CORES)], axis=0)
    return out.astype(np.float32), res


def _install_trace_shim():
    import types
    mod = types.ModuleType("antenv.axon_hooks")
    holder = [None]
    mod.set_axon_ntff_profile_hook = lambda h: holder.__setitem__(0, h)
    mod.get_axon_ntff_profile_hook = lambda: holder[0]
    sys.modules["antenv.axon_hooks"] = mod
    try:
        import antenv
        antenv.axon_hooks = mod
    except ImportError:
        pass
    try:
        from trn_agent_boot.trn_boot import _ntff_profile_via_ctypes
        mod.set_axon_ntff_profile_hook(
            _ntff_profile_via_ctypes("/opt/axon/libaxon_pjrt.so")
        )
    except Exception:
        pass
    bass_utils.upload_artifacts = lambda d: "local://skipped"


def kernel(**inputs) -> np.ndarray:
    out, _ = run(inputs, trace=bool(os.environ.get("KERNEL_TRACE")))
    return out

